# revision 1
# baseline (speedup 1.0000x reference)
"""GAT (2-layer) Trainium2 Bass kernel — 8-core SPMD, v2.

Sharding: dst nodes across 8 cores (12500 each). Per core, dsts are packed
into 98 windows of 128 (one SBUF partition per dst), sorted by per-group max
degree so slot padding is small (~1.45x). Layer-1 edge rows are fp8e3
[a_s 8xf32 | h (8 heads x 17) with a ones-lane per head] = 256B, fetched by
gpsimd.dma_gather from 4 src-groups of 25088 rows (int16 idx limit); the
ones-lane makes the softmax denominator fall out of the same weighted
reduction as the numerator. Gathers are pipelined: descriptor prep+trigger
on gpsimd, completion waited on the vector engine via rotating DMA
semaphores, so gather DMA overlaps DVE compute of the previous batch.
Attention lrelu/exp run on the scalar (ACT) engine. Pad slots point at a row
with a_s=-300 => weights ~e^-56. Layer-2 node table is built per-shard and
AllGathered into a Shared scratchpad.
"""

import os as _os

import numpy as np
import ml_dtypes

import concourse.bacc as bacc
import concourse.bass as bass
import concourse.mybir as mybir
import concourse.tile as tile
from concourse.bass_utils import run_bass_kernel_spmd
from concourse.masks import make_identity

F32 = mybir.dt.float32
BF16 = mybir.dt.bfloat16
F8 = mybir.dt.float8e3
I16 = mybir.dt.int16
AX = mybir.AxisListType
OP = mybir.AluOpType
ACT = mybir.ActivationFunctionType

N, E = 100000, 1600000
IN, HID, OUT, HEADS = 256, 16, 64, 8
NEG = 0.2
NCORES = 8
NSH = N // NCORES        # 12500
NGRP = 4
GSZ = N // NGRP          # 25000
NP = 25088               # padded rows per group (196*128)
NW = 98                  # windows per core
SH_ROWS = NW * 128       # 12544
PAD1 = GSZ               # group-local pad row, table1
PAD2 = NSH               # group-local pad row, table2 (12500 in core 2g's shard)
COLS_BUDGET = 104        # slot columns per gather batch
MAXW = 8                 # max windows per batch
ROW1 = 256               # fp8 elems per table1 row (256B)
ROW2 = 128               # bf16 elems per table2 row (256B)
NH1C = 17                # per-head lanes in L1 row: 16 ch + ones
NH2C = 65                # lanes in L2 row: 64 ch + ones
A_S_NEG = -300.0
NSEM = 4                 # rotating dma-completion semaphores


# ---------------------------------------------------------------- host side
def _layout(src, dst):
    core = dst // NSH
    grp = src // GSZ
    cg_all = np.zeros((NCORES, NSH, NGRP), np.int32)
    np.add.at(cg_all, (core, dst % NSH, grp), 1)
    perms = []
    for k in range(NCORES):
        cg = cg_all[k]
        mx = cg.max(axis=1)
        perms.append(np.lexsort(
            (cg[:, 3], cg[:, 2], cg[:, 1], cg[:, 0], mx))[::-1])
    Lg = np.zeros((NW, NGRP), np.int64)
    for k in range(NCORES):
        cgp = cg_all[k][perms[k]]
        cgp = np.concatenate([cgp, np.zeros((SH_ROWS - NSH, NGRP), np.int32)])
        Lg = np.maximum(Lg, cgp.reshape(NW, 128, NGRP).max(axis=1))
    Lw = Lg.sum(axis=1)
    sig = np.empty(N, np.int64)
    for k in range(NCORES):
        pos = np.empty(NSH, np.int64)
        pos[perms[k]] = np.arange(NSH)
        sig[k * NSH:(k + 1) * NSH] = k * SH_ROWS + pos
    eorder = np.lexsort((grp, dst))
    es, ed, eg, ec = src[eorder], dst[eorder], grp[eorder], core[eorder]
    core_starts = np.searchsorted(ec, np.arange(NCORES + 1))
    cores = [(es[a:b], (ed[a:b] - k * NSH), eg[a:b])
             for k, (a, b) in enumerate(zip(core_starts[:-1], core_starts[1:]))]
    return dict(Lg=Lg, Lw=Lw, perms=perms, sig=sig, cores=cores)


def _make_batches(Lw):
    batches = []
    cur, acc = [], 0
    for w in range(NW):
        lw = int(Lw[w])
        if cur and (acc + lw > COLS_BUDGET or len(cur) >= MAXW):
            batches.append(cur)
            cur, acc = [], 0
        cur.append(w)
        acc += lw
    if cur:
        batches.append(cur)
    return batches


def _pack_idx(arr_pj):
    """[128, cols] slot-array of indices -> wrapped idx tile [128, cols*8]."""
    I = arr_pj.T.ravel()                      # I[j*128+p]
    W = I.reshape(-1, 16).T.astype(np.int16)  # [16, len/16]
    return np.tile(W, (8, 1))


def _host_inputs(inputs, lay, batches):
    x = np.asarray(inputs["x"], np.float32)
    W1 = np.asarray(inputs["W1"], np.float64)
    att1_s = np.asarray(inputs["att1_s"], np.float64)
    att1_d = np.asarray(inputs["att1_d"], np.float64)
    W2 = np.asarray(inputs["W2"], np.float64)
    att2_s = np.asarray(inputs["att2_s"], np.float64)
    att2_d = np.asarray(inputs["att2_d"], np.float64)
    b1 = np.asarray(inputs["b1"], np.float32)
    b2 = np.asarray(inputs["b2"], np.float32)
    Lg, Lw, perms, sig = lay["Lg"], lay["Lw"], lay["perms"], lay["sig"]

    A_s = np.zeros((HEADS * HID, HEADS))
    A_d = np.zeros((HEADS * HID, HEADS))
    for h in range(HEADS):
        A_s[h * HID:(h + 1) * HID, h] = att1_s[h]
        A_d[h * HID:(h + 1) * HID, h] = att1_d[h]
    w1r = np.concatenate([W1, W1 @ A_s, W1 @ A_d], axis=1)          # [256,144]
    w2r = np.concatenate([W2, W2 @ att2_s.T, W2 @ att2_d.T], axis=1)  # [128,66]
    w1r_bf = w1r.astype(ml_dtypes.bfloat16)
    w2r_bf = w2r.astype(ml_dtypes.bfloat16)

    xT = np.zeros((IN, NGRP * NP), np.float32)
    for g in range(NGRP):
        xT[:, g * NP:g * NP + GSZ] = x[g * GSZ:(g + 1) * GSZ].T
    xT_bf = xT.astype(ml_dtypes.bfloat16)

    common = {
        "xt0": np.ascontiguousarray(xT_bf[:128]),
        "xt1": np.ascontiguousarray(xT_bf[128:]),
        "w1r0": np.ascontiguousarray(w1r_bf[:128]),
        "w1r1": np.ascontiguousarray(w1r_bf[128:]),
        "w2r": np.ascontiguousarray(w2r_bf),
        "b1rep": np.ascontiguousarray(np.tile(b1[None, :], (128, 1)).astype(np.float32)),
        "b2rep": np.ascontiguousarray(np.tile(b2[None, :], (128, 1)).astype(np.float32)),
    }

    per_core = []
    for k in range(NCORES):
        es, edl, eg = lay["cores"][k]
        pos = np.empty(NSH, np.int64)
        pos[perms[k]] = np.arange(NSH)
        o = np.lexsort((eg, pos[edl]))
        es_o, eg_o, pos_o = es[o], eg[o], pos[edl][o]
        w_o, p_o = pos_o // 128, pos_o % 128
        key = pos_o * NGRP + eg_o
        slot = np.arange(len(o)) - np.searchsorted(key, key)
        idx1_secs, idx2_secs = [], []
        for ws in batches:
            for g in range(NGRP):
                cols = int(Lg[ws, g].sum())
                if cols == 0:
                    continue
                a1 = np.full((128, cols), PAD1, np.int64)
                a2 = np.full((128, cols), PAD2, np.int64)
                coff = 0
                for w in ws:
                    m = (w_o == w) & (eg_o == g)
                    pp, jj, ss = p_o[m], slot[m], es_o[m]
                    a1[pp, coff + jj] = ss % GSZ
                    a2[pp, coff + jj] = sig[ss] % NP
                    coff += int(Lg[w, g])
                idx1_secs.append(a1)
                idx2_secs.append(a2)
        idx1 = np.concatenate([_pack_idx(a) for a in idx1_secs], axis=1)
        idx2 = np.concatenate([_pack_idx(a) for a in idx2_secs], axis=1)
        xtp = np.zeros((IN, SH_ROWS), np.float32)
        xtp[:, :NSH] = x[k * NSH:(k + 1) * NSH].T[:, perms[k]]
        xtp_bf = xtp.astype(ml_dtypes.bfloat16)
        d = dict(common)
        d["idx1"] = np.ascontiguousarray(idx1)
        d["idx2"] = np.ascontiguousarray(idx2)
        d["xtp0"] = np.ascontiguousarray(xtp_bf[:128])
        d["xtp1"] = np.ascontiguousarray(xtp_bf[128:])
        per_core.append(d)
    return per_core


# ------------------------------------------------------------- device side
def _build_program(Lg, Lw, batches):
    nc = bacc.Bacc("TRN2", target_bir_lowering=False, debug=False,
                   num_devices=NCORES, dynamic_dma_scratch_size=32768)
    IDXF = int(Lg.sum()) * 8
    LWMAX = int(Lw.max())
    MAXC = max(COLS_BUDGET, LWMAX)
    xt0 = nc.declare_dram_parameter("xt0", [128, NGRP * NP], BF16, isOutput=False)
    xt1 = nc.declare_dram_parameter("xt1", [128, NGRP * NP], BF16, isOutput=False)
    w1r0 = nc.declare_dram_parameter("w1r0", [128, 144], BF16, isOutput=False)
    w1r1 = nc.declare_dram_parameter("w1r1", [128, 144], BF16, isOutput=False)
    w2r = nc.declare_dram_parameter("w2r", [128, 66], BF16, isOutput=False)
    b1rep = nc.declare_dram_parameter("b1rep", [128, 128], F32, isOutput=False)
    b2rep = nc.declare_dram_parameter("b2rep", [128, 64], F32, isOutput=False)
    idx1 = nc.declare_dram_parameter("idx1", [128, IDXF], I16, isOutput=False)
    idx2 = nc.declare_dram_parameter("idx2", [128, IDXF], I16, isOutput=False)
    xtp0 = nc.declare_dram_parameter("xtp0", [128, SH_ROWS], BF16, isOutput=False)
    xtp1 = nc.declare_dram_parameter("xtp1", [128, SH_ROWS], BF16, isOutput=False)
    outp = nc.declare_dram_parameter("out", [SH_ROWS, OUT], F32, isOutput=True)

    table1 = nc.dram_tensor("table1", [NGRP * NP, ROW1], F8)
    shard2 = nc.dram_tensor("shard2", [SH_ROWS, ROW2], BF16)
    table2 = nc.dram_tensor("table2", [NCORES * SH_ROWS, ROW2], BF16,
                            addr_space="Shared")

    prep_sem = nc.alloc_semaphore("g_prep")
    dsems = [nc.alloc_semaphore(f"g_dma{i}") for i in range(NSEM)]
    cc_sem = nc.alloc_semaphore("cc")
    gn = [0]          # total gathers prepped
    bglob = [0]       # global batch counter (for sem rotation)
    dtarget = [0] * NSEM

    TPB, BLK, WRB = 196, 14, 7

    with tile.TileContext(nc) as tc:
        with (
            tc.tile_pool(name="const", bufs=1) as constp,
            tc.tile_pool(name="xt", bufs=2) as xtpool,
            tc.tile_pool(name="dense", bufs=2) as densep,
            tc.tile_pool(name="psum", bufs=2, space="PSUM") as psump,
            tc.tile_pool(name="stag", bufs=2) as stagp,
            tc.tile_pool(name="idx", bufs=2) as idxp,
            tc.tile_pool(name="work", bufs=1) as workp,
            tc.tile_pool(name="xeng", bufs=2) as xengp,
            tc.tile_pool(name="small", bufs=2) as smallp,
        ):
            w1r0_t = constp.tile([128, 144], BF16, tag="w1r0")
            w1r1_t = constp.tile([128, 144], BF16, tag="w1r1")
            w2r_t = constp.tile([128, 66], BF16, tag="w2r")
            b1_t = constp.tile([128, 128], F32, tag="b1")
            b2_t = constp.tile([128, 64], F32, tag="b2")
            ident = constp.tile([128, 128], BF16, tag="ident")
            adwin = constp.tile([128, NW * HEADS], F32, tag="adwin")
            ad2win = constp.tile([128, NW], F32, tag="ad2win")
            nc.sync.dma_start(out=w1r0_t[:], in_=w1r0[:])
            nc.sync.dma_start(out=w1r1_t[:], in_=w1r1[:])
            nc.sync.dma_start(out=w2r_t[:], in_=w2r[:])
            nc.sync.dma_start(out=b1_t[:], in_=b1rep[:])
            nc.sync.dma_start(out=b2_t[:], in_=b2rep[:])
            make_identity(nc, ident[:])

            # ---------------- phase 0: fp8 h1 table (all nodes) ------------
            for g in range(NGRP):
                for blk in range(TPB // BLK):
                    base = g * NP + blk * BLK * 128
                    xs0 = xtpool.tile([128, BLK * 128], BF16, tag="xs0")
                    xs1 = xtpool.tile([128, BLK * 128], BF16, tag="xs1")
                    nc.sync.dma_start(out=xs0[:], in_=xt0[:, base:base + BLK * 128])
                    nc.sync.dma_start(out=xs1[:], in_=xt1[:, base:base + BLK * 128])
                    for wb in range(BLK // WRB):
                        rows = densep.tile([128, WRB * ROW1], F8, tag="rows")
                        nc.vector.memset(rows[:], 1.0)
                        for t in range(WRB):
                            tt = wb * WRB + t
                            ps = psump.tile([128, 144], F32, tag="ps0")
                            nc.tensor.matmul(
                                out=ps[:], lhsT=xs0[:, tt * 128:(tt + 1) * 128],
                                rhs=w1r0_t[:], start=True, stop=False)
                            nc.tensor.matmul(
                                out=ps[:], lhsT=xs1[:, tt * 128:(tt + 1) * 128],
                                rhs=w1r1_t[:], start=False, stop=True)
                            rv = rows[:, t * ROW1:(t + 1) * ROW1]
                            nc.vector.tensor_copy(out=rv[:, 0:32].bitcast(F32),
                                                  in_=ps[:, 128:136])
                            nc.vector.tensor_copy(
                                out=rv[:, 32:32 + HEADS * NH1C]
                                    .rearrange("p (h c) -> p h c", h=HEADS)
                                    [:, :, 0:HID],
                                in_=ps[:, 0:128]
                                    .rearrange("p (h c) -> p h c", h=HEADS))
                        wbase = g * NP + (blk * BLK + wb * WRB) * 128
                        nc.sync.dma_start(
                            out=table1[wbase:wbase + WRB * 128, :]
                                .rearrange("(a p) r -> p a r", p=128),
                            in_=rows[:].rearrange("p (a r) -> p a r", a=WRB))
            # pad row: all-zero (ones-lane too), a_s := -300
            padrow = constp.tile([128, ROW1], F8, tag="padrow")
            nc.vector.memset(padrow[:], 0.0)
            nc.vector.memset(padrow[0:1, 0:32].bitcast(F32), A_S_NEG)
            for g in range(NGRP):
                nc.sync.dma_start(out=table1[g * NP + PAD1:g * NP + PAD1 + 1, :],
                                  in_=padrow[0:1, :])

            # a_d per window (window-ordered x.T), 7 windows per block
            ADB = 7
            for wb in range(NW // ADB):
                w0 = wb * ADB
                xp0 = xtpool.tile([128, ADB * 128], BF16, tag="xp0")
                xp1 = xtpool.tile([128, ADB * 128], BF16, tag="xp1")
                nc.sync.dma_start(out=xp0[:],
                                  in_=xtp0[:, w0 * 128:(w0 + ADB) * 128])
                nc.sync.dma_start(out=xp1[:],
                                  in_=xtp1[:, w0 * 128:(w0 + ADB) * 128])
                psa = psump.tile([128, ADB * 16], F32, tag="psa")
                for t in range(ADB):
                    nc.tensor.matmul(out=psa[:, t * 16:(t + 1) * 16],
                                     lhsT=xp0[:, t * 128:(t + 1) * 128],
                                     rhs=w1r0_t[:, 128:144],
                                     start=True, stop=False)
                    nc.tensor.matmul(out=psa[:, t * 16:(t + 1) * 16],
                                     lhsT=xp1[:, t * 128:(t + 1) * 128],
                                     rhs=w1r1_t[:, 128:144],
                                     start=False, stop=True)
                nc.vector.tensor_copy(
                    out=adwin[:, w0 * 8:(w0 + ADB) * 8]
                        .rearrange("p (a h) -> p a h", a=ADB),
                    in_=psa[:].rearrange("p (a h) -> p a h", a=ADB)[:, :, 8:16])

            # ---------------- edge layers ----------------------------------
            def edge_layer(layer):
                NBATCH = int(_os.environ.get("GAT_NBATCH", "999"))
                idxin = idx1 if layer == 1 else idx2
                nh = HEADS if layer == 1 else 1
                nch = HID if layer == 1 else OUT
                nlan = NH1C if layer == 1 else NH2C   # lanes per head incl ones
                idx_off = [0]

                def gather_batch(ws, prev):
                    nw = len(ws)
                    cols_b = int(Lw[ws].sum())
                    gcols = [int(Lg[ws, g].sum()) for g in range(NGRP)]
                    gbase = np.concatenate([[0], np.cumsum(gcols)])
                    sem_i = bglob[0] % NSEM
                    bglob[0] += 1

                    ixt = idxp.tile([128, MAXC * 8], I16, tag="ix")
                    nc.sync.dma_start(
                        out=ixt[:, 0:cols_b * 8],
                        in_=idxin[:, idx_off[0]:idx_off[0] + cols_b * 8])
                    idx_off[0] += cols_b * 8
                    stag = stagp.tile([128, MAXC * 256], F8, tag="st")
                    ng = sum(1 for g in range(NGRP) if gcols[g] > 0)
                    with tc.tile_critical():
                        for g in range(NGRP):
                            cols = gcols[g]
                            if cols == 0:
                                continue
                            nidx = 128 * cols
                            if layer == 1:
                                sl3 = stag[:, int(gbase[g]) * 256:
                                           (int(gbase[g]) + cols) * 256] \
                                    .rearrange("p (k d) -> p k d", d=256)
                                in_ap = table1[g * NP:(g + 1) * NP, :]
                                esz = ROW1
                            else:
                                sl3 = stag.bitcast(BF16)[
                                    :, int(gbase[g]) * 128:
                                    (int(gbase[g]) + cols) * 128] \
                                    .rearrange("p (k d) -> p k d", d=128)
                                in_ap = table2[g * NP:(g + 1) * NP, :]
                                esz = ROW2
                            gn[0] += 1
                            nc.gpsimd.dma_gather(
                                out_ap=sl3, in_ap=in_ap,
                                idxs_ap=ixt[:, int(gbase[g]) * 8:
                                            int(gbase[g]) * 8 + nidx // 16],
                                num_idxs=nidx, num_idxs_reg=nidx,
                                elem_size=esz, single_packet=False,
                                prepare_only=True, sem=dsems[sem_i],
                            ).then_inc(prep_sem, 1)
                        nc.gpsimd.wait_ge(prep_sem, gn[0])
                        nc.gpsimd.trigger_dma(count=ng)
                        if prev is not None:
                            nc.vector.wait_ge(dsems[prev["sem_i"]],
                                              prev["target"])
                            nc.vector.memset(prev["stag"][0:1, 200:201], 0.0)
                    dtarget[sem_i] += 16 * ng
                    return dict(ws=ws, nw=nw, cols_b=cols_b, gbase=gbase,
                                stag=stag, sem_i=sem_i, target=dtarget[sem_i])

                def final_wait(prev):
                    with tc.tile_critical():
                        nc.vector.wait_ge(dsems[prev["sem_i"]], prev["target"])
                        nc.vector.memset(prev["stag"][0:1, 200:201], 0.0)

                def compute_batch(stt):
                    ws, nw, cols_b = stt["ws"], stt["nw"], stt["cols_b"]
                    gbase, stag = stt["gbase"], stt["stag"]

                    CLEVEL = int(_os.environ.get("GAT_CLEVEL", "4"))
                    if CLEVEL == 0:
                        dbg = smallp.tile([128, 8], F32, tag="dbg")
                        nc.vector.tensor_copy(out=dbg[:],
                                              in_=stag[:, 0:32].bitcast(F32))
                        return

                    # attention weights ew = exp(lrelu(a_s + a_d))
                    ew = workp.tile([128, MAXC * HEADS], F32, tag="ew")
                    for g in range(NGRP):
                        coff = 0
                        for w in ws:
                            Lgv = int(Lg[w, g])
                            if Lgv == 0:
                                continue
                            c0 = int(gbase[g]) + coff
                            if layer == 1:
                                a_s = stag[:, c0 * 256:(c0 + Lgv) * 256] \
                                    .rearrange("p (k d) -> p k d", d=256) \
                                    [:, :, 0:32].bitcast(F32)
                            else:
                                a_s = stag[:, c0 * 256:(c0 + Lgv) * 256] \
                                    .rearrange("p (k d) -> p k d", d=256) \
                                    [:, :, 0:4].bitcast(F32)
                            adv = (adwin[:, w * 8:(w + 1) * 8] if layer == 1
                                   else ad2win[:, w:w + 1]) \
                                .rearrange("p (l h) -> p l h", l=1) \
                                .to_broadcast([128, Lgv, nh])
                            uv = ew[:, c0 * nh:(c0 + Lgv) * nh] \
                                .rearrange("p (l h) -> p l h", l=Lgv)
                            nc.vector.tensor_tensor(out=uv, in0=a_s, in1=adv,
                                                    op=OP.add)
                            coff += Lgv
                    ewv = ew[:, 0:cols_b * nh]
                    lr = workp.tile([128, MAXC * HEADS], F32, tag="lr")
                    nc.vector.tensor_scalar_mul(lr[:, 0:cols_b * nh], ewv, NEG)
                    nc.vector.tensor_tensor(out=ewv, in0=ewv,
                                            in1=lr[:, 0:cols_b * nh], op=OP.max)
                    nc.scalar.activation(ewv, ewv, ACT.Exp, 0.0, 1.0)
                    if CLEVEL == 1:
                        return

                    # weighted messages (ones-lane gives denominator)
                    nhl = nh * nlan
                    msg = workp.tile([128, MAXC * HEADS * NH1C], BF16, tag="mg")
                    if layer == 1:
                        hv = stag[:, 0:cols_b * 256] \
                            .rearrange("p (k d) -> p k d", d=256) \
                            [:, :, 32:32 + nhl] \
                            .rearrange("p k (h c) -> p k h c", h=nh)
                    else:
                        hv = stag.bitcast(BF16)[:, 0:cols_b * 128] \
                            .rearrange("p (k d) -> p k d", d=128) \
                            [:, :, 2:2 + nhl] \
                            .rearrange("p k (h c) -> p k h c", h=nh)
                    nc.vector.tensor_tensor(
                        out=msg[:, 0:cols_b * nhl]
                            .rearrange("p (k h c) -> p k h c", h=nh, c=nlan),
                        in0=hv,
                        in1=ewv.rearrange("p (k h c) -> p k h c", h=nh, c=1)
                            .to_broadcast([128, cols_b, nh, nlan]),
                        op=OP.mult)

                    # segment sums: per (w,g) partial then combine over g
                    op4 = workp.tile([128, MAXW * NGRP * HEADS * NH1C], F32,
                                     tag="op4")
                    nc.vector.memset(op4[:, 0:nw * NGRP * nhl], 0.0)
                    for g in range(NGRP):
                        coff = 0
                        for wi, w in enumerate(ws):
                            Lgv = int(Lg[w, g])
                            if Lgv == 0:
                                continue
                            c0 = int(gbase[g]) + coff
                            nc.vector.tensor_reduce(
                                out=op4[:, (wi * NGRP + g) * nhl:
                                        (wi * NGRP + g + 1) * nhl],
                                in_=msg[:, c0 * nhl:(c0 + Lgv) * nhl]
                                    .rearrange("p (l c) -> p c l", c=nhl),
                                axis=AX.X, op=OP.add)
                            coff += Lgv
                    opr = workp.tile([128, MAXW * HEADS * NH1C], F32, tag="opr")
                    nc.vector.tensor_reduce(
                        out=opr[:, 0:nw * nhl],
                        in_=op4[:, 0:nw * NGRP * nhl]
                            .rearrange("p (w g c) -> p w c g", w=nw, g=NGRP),
                        axis=AX.X, op=OP.add)
                    if CLEVEL == 2:
                        return

                    # normalize: o1 = num / den  (den = ones-lane)
                    o3 = opr[:, 0:nw * nhl].rearrange(
                        "p (w h c) -> p w h c", w=nw, h=nh)
                    den = smallp.tile([128, MAXW * HEADS], F32, tag="den")
                    nc.vector.tensor_scalar_max(
                        den[:, 0:nw * nh].rearrange(
                            "p (w h c) -> p w h c", w=nw, h=nh, c=1),
                        o3[:, :, :, nch:nch + 1], 1e-30)
                    rec = smallp.tile([128, MAXW * HEADS], F32, tag="rec")
                    nc.vector.reciprocal(rec[:, 0:nw * nh], den[:, 0:nw * nh])
                    o1 = workp.tile([128, MAXW * 128], F32, tag="o1")
                    nc.vector.tensor_tensor(
                        out=o1[:, 0:nw * nh * nch]
                            .rearrange("p (w h c) -> p w h c", w=nw, h=nh),
                        in0=o3[:, :, :, 0:nch],
                        in1=rec[:, 0:nw * nh]
                            .rearrange("p (w h c) -> p w h c", w=nw, h=nh, c=1)
                            .to_broadcast([128, nw, nh, nch]),
                        op=OP.mult)

                    if layer == 1:
                        o1v = o1[:, 0:nw * 128]
                        o1r = o1v.rearrange("p (w c) -> p w c", w=nw)
                        nc.vector.tensor_tensor(
                            out=o1r, in0=o1r,
                            in1=b1_t[:].rearrange("p (w c) -> p w c", w=1)
                                .to_broadcast([128, nw, 128]),
                            op=OP.add)
                        tneg = workp.tile([128, MAXW * 128], F32, tag="tneg")
                        nc.vector.tensor_scalar_min(tneg[:, 0:nw * 128], o1v, 0.0)
                        nc.scalar.activation(tneg[:, 0:nw * 128],
                                             tneg[:, 0:nw * 128], ACT.Exp,
                                             0.0, 1.0)
                        nc.vector.tensor_relu(o1v, o1v)
                        nc.vector.tensor_tensor(out=o1v, in0=o1v,
                                                in1=tneg[:, 0:nw * 128],
                                                op=OP.add)
                        nc.vector.tensor_scalar_add(o1v, o1v, -1.0)
                        o1bf = xengp.tile([128, MAXW * 128], BF16, tag="o1bf")
                        nc.vector.tensor_copy(out=o1bf[:, 0:nw * 128], in_=o1v)
                        if CLEVEL == 3:
                            return
                        row2 = xengp.tile([128, MAXW * 128], BF16, tag="row2")
                        for wi, w in enumerate(ws):
                            pst = psump.tile([128, 128], BF16, tag="pst")
                            nc.tensor.transpose(
                                out=pst[:],
                                in_=o1bf[:, wi * 128:(wi + 1) * 128],
                                identity=ident[:])
                            o1T = xengp.tile([128, 128], BF16, tag="o1T")
                            nc.vector.tensor_copy(out=o1T[:], in_=pst[:])
                            ps2 = psump.tile([128, 66], F32, tag="ps2")
                            nc.tensor.matmul(out=ps2[:], lhsT=o1T[:],
                                             rhs=w2r_t[:], start=True, stop=True)
                            rv = row2[:, wi * 128:(wi + 1) * 128]
                            nc.vector.tensor_copy(out=rv[:, 0:2].bitcast(F32),
                                                  in_=ps2[:, 64:65])
                            nc.vector.tensor_copy(out=rv[:, 2:66],
                                                  in_=ps2[:, 0:64])
                            nc.vector.tensor_copy(out=ad2win[:, w:w + 1],
                                                  in_=ps2[:, 65:66])
                        nc.vector.memset(
                            row2[:, 0:nw * 128]
                                .rearrange("p (w c) -> p w c", w=nw)
                                [:, :, 66:67], 1.0)
                        w0 = ws[0]
                        nc.sync.dma_start(
                            out=shard2[w0 * 128:(w0 + nw) * 128, :]
                                .rearrange("(a p) r -> p a r", p=128),
                            in_=row2[:, 0:nw * 128]
                                .rearrange("p (a r) -> p a r", a=nw))
                    else:
                        o1v = o1[:, 0:nw * 64]
                        o1r = o1v.rearrange("p (w c) -> p w c", w=nw)
                        nc.vector.tensor_tensor(
                            out=o1r, in0=o1r,
                            in1=b2_t[:].rearrange("p (w c) -> p w c", w=1)
                                .to_broadcast([128, nw, 64]),
                            op=OP.add)
                        mx = smallp.tile([128, MAXW], F32, tag="mx")
                        nc.vector.tensor_reduce(
                            out=mx[:, 0:nw],
                            in_=o1r, axis=AX.X, op=OP.max)
                        sh = workp.tile([128, MAXW * 64], F32, tag="sh")
                        shr = sh[:, 0:nw * 64].rearrange("p (w c) -> p w c", w=nw)
                        nc.vector.tensor_tensor(
                            out=shr, in0=o1r,
                            in1=mx[:, 0:nw]
                                .rearrange("p (w c) -> p w c", c=1)
                                .to_broadcast([128, nw, 64]),
                            op=OP.subtract)
                        ex = workp.tile([128, MAXW * 64], F32, tag="ex")
                        nc.scalar.activation(ex[:, 0:nw * 64], sh[:, 0:nw * 64],
                                             ACT.Exp, 0.0, 1.0)
                        se = smallp.tile([128, MAXW], F32, tag="se")
                        nc.vector.tensor_reduce(
                            out=se[:, 0:nw],
                            in_=ex[:, 0:nw * 64]
                                .rearrange("p (w c) -> p w c", w=nw),
                            axis=AX.X, op=OP.add)
                        ln = smallp.tile([128, MAXW], F32, tag="ln")
                        nc.scalar.activation(ln[:, 0:nw], se[:, 0:nw],
                                             ACT.Ln, 0.0, 1.0)
                        fo = xengp.tile([128, MAXW * 64], F32, tag="fo")
                        nc.vector.tensor_tensor(
                            out=fo[:, 0:nw * 64]
                                .rearrange("p (w c) -> p w c", w=nw),
                            in0=shr,
                            in1=ln[:, 0:nw]
                                .rearrange("p (w c) -> p w c", c=1)
                                .to_broadcast([128, nw, 64]),
                            op=OP.subtract)
                        if CLEVEL == 3:
                            return
                        w0 = ws[0]
                        nc.sync.dma_start(
                            out=outp[w0 * 128:(w0 + nw) * 128, :]
                                .rearrange("(a p) r -> p a r", p=128),
                            in_=fo[:, 0:nw * 64]
                                .rearrange("p (a r) -> p a r", a=nw))

                prev = None
                for ws in batches[:NBATCH]:
                    cur = gather_batch(ws, prev)
                    if prev is not None:
                        compute_batch(prev)
                    prev = cur
                if prev is not None:
                    final_wait(prev)
                    compute_batch(prev)

            STAGE = int(_os.environ.get("GAT_STAGE", "3"))
            if STAGE >= 1:
                edge_layer(1)
            pr2 = constp.tile([1, 2], BF16, tag="pr2")
            nc.vector.memset(pr2[0:1, 0:2].bitcast(F32), A_S_NEG)
            nc.sync.dma_start(out=shard2[PAD2:PAD2 + 1, 0:2], in_=pr2[0:1, :])
            if STAGE >= 2:
                with tc.tile_critical():
                    nc.gpsimd.collective_compute(
                        "AllGather", OP.bypass,
                        replica_groups=[list(range(NCORES))],
                        ins=[shard2[:]], outs=[table2[:]],
                    ).then_inc(cc_sem, 1)
                    nc.gpsimd.wait_ge(cc_sem, 1)
            if STAGE >= 3:
                edge_layer(2)
    nc.compile()
    return nc


_CACHE = {}


def kernel(**inputs):
    ei = np.asarray(inputs["edge_index"])
    src, dst = ei[0].astype(np.int64), ei[1].astype(np.int64)
    lay = _layout(src, dst)
    batches = _make_batches(lay["Lw"])
    per_core = _host_inputs(inputs, lay, batches)
    key = (ei.tobytes()[:64], int(lay["Lg"].sum()))
    if key not in _CACHE:
        _CACHE[key] = _build_program(lay["Lg"], lay["Lw"], batches)
    nc = _CACHE[key]
    res = run_bass_kernel_spmd(nc, per_core, core_ids=list(range(NCORES)))
    out = np.empty((N, OUT), np.float32)
    for k in range(NCORES):
        out[k * NSH + lay["perms"][k]] = res.results[k]["out"][:NSH]
    return out


if __name__ == "__main__":
    d = np.load("/root/problem/_inp_check.npz")
    o = kernel(**{k: d[k] for k in d.files})
    ref = np.load("/root/problem/_ref_check.npy")
    rel = np.linalg.norm(o - ref) / np.linalg.norm(ref)
    err = np.abs(o - ref) / (np.abs(ref) + 1e-5)
    print("fro rel err:", rel, "max elem rel err:", err.max())

